# revision 31
# baseline (speedup 1.0000x reference)
"""Trainium2 Bass kernel for nn_DeformLayer (deformable conv block).

Sharding: data-parallel over batch, 1 sample per NeuronCore across 8 cores.

Per-core pipeline:
  offset conv (PE, bf16) -> PE-transpose om to position-major -> bilinear
  fields (DVE fp32) -> wrapped-index build fully on-chip (PE transposes of
  the int16 index tile into the 16-partition wrapped layout, replicated x8
  for the gpsimd cores) -> ONE pair dma_gather per 128-position block from
  HBM xT2 (bf16; each descriptor fetches the full 2x2 bilinear patch:
  1024 ch-elems covering (y0,y0+1)x(x0,x0+1)) -> factorized coefficient
  combine (DVE tensor_scalar per-k with per-partition scalar APs at 4x +
  two batched pair-adds at 2x) -> dma_start_transpose to channel-major ->
  DCN matmul (PE) -> BN1+ReLU (ACT) -> 4-parity 2x2 deconv (PE)
  -> BN2+ReLU (ACT) -> out [256, 128, 128] fp32.
"""
import numpy as np
import ml_dtypes
from contextlib import ExitStack

import concourse.bass as bass
import concourse.tile as tile
from concourse import bacc, mybir
from concourse import bass_utils
from concourse.library_config import mlp

BF16 = ml_dtypes.bfloat16
F32 = mybir.dt.float32
BF = mybir.dt.bfloat16
I16 = mybir.dt.int16
I32 = mybir.dt.int32
AL = mybir.AluOpType
AF = mybir.ActivationFunctionType

def _register_wpair():
    """Register the fused weighted-pair-add custom DVE op (out = a*s0 + b*s1).

    Uses the documented extension point (dve_ops.OPS); idempotent so repeated
    kernel imports are safe. uops_sha is computed from lower() so the pinned
    hashes always match this concourse build.
    """
    from concourse import dve_ops as DO
    from concourse.dve_spec import Spec, Src0, Src1, C0, C1, lower
    from concourse.dve_uop import DveOpSpec

    name = "WPAIR_ANT"
    for op in DO.OPS:
        if op.name == name:
            return op
    spec = Spec(body=Src0 * C0 + Src1 * C1,
                reference=lambda in0, in1, s0, s1, imm2=0.0: in0 * s0 + in1 * s1)
    row = DO._CUSTOM_DVE_ROW_BASE + len(DO.OPS)
    sha = {}
    for ver in ("v3", "v4"):
        u = lower(spec, ver=ver)
        sha[ver] = DveOpSpec(name=name, opcode=row, uops=u, rd1_en=True).sha(ver)
    op = DO.DveOp(name, spec, subdim=False, uops_sha=sha)
    DO.OPS.append(op)
    DO.CUSTOM_DVE_SPECS[name] = spec
    DO._SUB_OPCODE_FOR_NAME[name] = row
    return op


WPAIR = _register_wpair()

EPS = 1e-5
H = W = 64
HW = H * W          # 4096
C = Co = 256
NHT = 16            # half-tiles of 256 positions
NBLK = 32           # 128-position blocks
PADHW = 66 * 66     # 4356
NROW2 = 65 * 64     # xT2 rows: (y+1) in [0,64] x 64 cols

TAP0 = [(1, 0), (3, -1)]   # parity 0: (kh/kw, shift)
TAP1 = [(0, 1), (2, 0)]    # parity 1


def _ap(base, off, dims):
    return bass.AP(base.tensor, base.offset + off, [list(d) for d in dims])


def build_nc():
    nc = bacc.Bacc("TRN2", target_bir_lowering=False, debug=False,
                   num_devices=8, num_swdge_queues=4)

    d_xpad = nc.dram_tensor("xpad", [2, 128, PADHW], BF, kind="ExternalInput")
    d_xT2 = nc.dram_tensor("xT2", [NROW2 * 512], BF, kind="ExternalInput")
    d_FB = nc.dram_tensor("FB", [128, NBLK, 27], F32, kind="ExternalInput")
    d_woff = nc.dram_tensor("woff", [9, 2, 128, 27], BF, kind="ExternalInput")
    d_wdcn = nc.dram_tensor("wdcn", [128, 18, 256], BF, kind="ExternalInput")
    d_wup = nc.dram_tensor("wup", [128, 4, 8, 256], BF, kind="ExternalInput")
    d_bn1 = nc.dram_tensor("bn1", [2, 2, 128], F32, kind="ExternalInput")
    d_bn2 = nc.dram_tensor("bn2", [2, 2, 128], F32, kind="ExternalInput")
    d_id27 = nc.dram_tensor("id27", [27, 27], F32, kind="ExternalInput")
    d_id128 = nc.dram_tensor("id128", [128, 128], F32, kind="ExternalInput")
    d_out = nc.dram_tensor("out", [256, 128, 128], BF, kind="ExternalOutput")

    with tile.TileContext(nc) as tc, ExitStack() as ctx:
        p_const = ctx.enter_context(tc.tile_pool(name="const", bufs=1))
        p_head = ctx.enter_context(tc.tile_pool(name="head", bufs=1))
        p_ftmp = ctx.enter_context(tc.tile_pool(name="ftmp", bufs=1))
        p_om = ctx.enter_context(tc.tile_pool(name="om", bufs=2))
        p_ps_om = ctx.enter_context(tc.tile_pool(name="psom", bufs=2, space="PSUM"))
        p_ps_ix = ctx.enter_context(tc.tile_pool(name="psix", bufs=1, space="PSUM"))
        p_G = ctx.enter_context(tc.tile_pool(name="G", bufs=3))
        p_gT = ctx.enter_context(tc.tile_pool(name="gT", bufs=2))
        p_gall = ctx.enter_context(tc.tile_pool(name="gall", bufs=3))
        p_ps_dcn = ctx.enter_context(tc.tile_pool(name="psdcn", bufs=2, space="PSUM"))
        p_ps_dc = ctx.enter_context(tc.tile_pool(name="psdc", bufs=2, space="PSUM"))
        p_outst = ctx.enter_context(tc.tile_pool(name="outst", bufs=4))

        nc.gpsimd.load_library(mlp)

        # ---------------- constant loads ----------------
        woff_sb = p_const.tile([128, 18, 27], BF)
        nc.sync.dma_start(
            woff_sb[:], _ap(d_woff.ap(), 0, [[27, 128], [128 * 27, 18], [1, 27]]))
        wdcn_sb = p_const.tile([128, 18, 256], BF)
        nc.sync.dma_start(wdcn_sb[:], d_wdcn.ap())
        wup_sb = p_const.tile([128, 4, 8, 256], BF)
        nc.sync.dma_start(wup_sb[:], d_wup.ap())
        bn1_sb = p_const.tile([128, 2, 2], F32)  # [o%128][s/b][ohalf]
        nc.sync.dma_start(
            bn1_sb[:], _ap(d_bn1.ap(), 0, [[1, 128], [256, 2], [128, 2]]))
        bn2_sb = p_const.tile([128, 2, 2], F32)
        nc.sync.dma_start(
            bn2_sb[:], _ap(d_bn2.ap(), 0, [[1, 128], [256, 2], [128, 2]]))
        id27_sb = p_const.tile([27, 27], F32)
        nc.sync.dma_start(id27_sb[:], d_id27.ap())
        id128_sb = p_const.tile([128, 128], F32)
        nc.sync.dma_start(id128_sb[:], d_id128.ap())
        FB_sb = p_const.tile([128, NBLK, 27], F32)
        nc.sync.dma_start(FB_sb[:], d_FB.ap())
        xpad_sb = p_head.tile([128, 2, PADHW], BF)
        nc.sync.dma_start(
            xpad_sb[:],
            _ap(d_xpad.ap(), 0, [[PADHW, 128], [128 * PADHW, 2], [1, PADHW]]))

        # deconv input bands: [128(o%128), td(8), ohalf(2), 10*66] bf16, zeroed
        bands = p_const.tile([128, 8, 2, 660], BF)
        nc.vector.memset(bands[:], 0.0)

        # -------- pipelined preamble: per cnk (8 blocks = 2 N-tiles): --------
        # offset conv -> om transpose -> field slice -> wrapped-index build.
        # Each cnk writes its own gidx tile so the main loop's gathers for
        # h in [4*cnk, 4*cnk+4) start as soon as that quarter is ready.
        NF = NBLK * 9  # 288
        omT = p_head.tile([128, NBLK, 27], F32)

        def f9c(tt, j0, cnk):  # [128, 8blk, 9] view at channel j0, cnk slice
            return _ap(tt[:], j0 + cnk * 8 * 27,
                       [[NBLK * 27, 128], [27, 8], [1, 9]])

        def ftile(tag, dt=F32):
            return p_ftmp.tile([128, NF], dt, tag=tag, name=tag)

        px = ftile("px"); py = ftile("py"); mask = ftile("mask")
        x0 = ftile("x0"); y0 = ftile("y0"); ti32 = ftile("i32tmp", I32)
        wy0m = ftile("wy0m"); wy1m = ftile("wy1m"); xp = ftile("xp")
        ta = ftile("ta"); tb = ftile("tb"); tc_ = ftile("tc"); td = ftile("td")
        cs0 = ftile("cs0"); cs1 = ftile("cs1")
        vidx = p_head.tile([128, NF], F32)
        id72 = _ap(id128_sb[:], 0, [[128, 72], [1, 72]])
        gidxs = [p_head.tile([128, 8, 72], I16, tag=f"gidx{c}", name=f"gidx{c}")
                 for c in range(4)]

        for cnk in range(4):
            # ---- offset conv for nt = 2*cnk, 2*cnk+1 ----
            for nt in (2 * cnk, 2 * cnk + 1):
                ps = p_ps_om.tile([27, 512], F32, tag="psom", name="psom")
                first = True
                for k in range(9):
                    ky, kx = k // 3, k % 3
                    for cb in range(2):
                        rhs = _ap(xpad_sb[:],
                                  cb * PADHW + (nt * 8 + ky) * 66 + kx,
                                  [[2 * PADHW, 128], [66, 8], [1, 64]])
                        nc.tensor.matmul(ps[:], woff_sb[:, k * 2 + cb, :], rhs,
                                         start=first, stop=(k == 8 and cb == 1))
                        first = False
                om_nt = p_om.tile([27, 512], F32, tag="omnt", name="omnt")
                nc.scalar.copy(om_nt[:], ps[:])
                for i in range(4):
                    pst = p_ps_ix.tile([128, 72], F32, tag="pscm", name="pscm")
                    nc.tensor.transpose(pst[:, 0:27],
                                        om_nt[:, i * 128:(i + 1) * 128],
                                        id27_sb[:])
                    nc.scalar.copy(omT[:, nt * 4 + i, :], pst[:, 0:27])

            # ---- bilinear fields on this cnk's 72-column slice ----
            sl = slice(cnk * 72, (cnk + 1) * 72)

            nc.vector.tensor_tensor(px[:, sl], f9c(omT, 0, cnk),
                                    f9c(FB_sb, 0, cnk), AL.add)
            nc.vector.tensor_tensor(py[:, sl], f9c(omT, 9, cnk),
                                    f9c(FB_sb, 9, cnk), AL.add)
            nc.vector.tensor_tensor(mask[:, sl], f9c(omT, 18, cnk),
                                    f9c(FB_sb, 18, cnk), AL.add)
            nc.scalar.activation(mask[:, sl], mask[:, sl], AF.Sigmoid)

            # floor, robust to cast rounding: f = cast(v); f -= (f > v)
            nc.vector.tensor_copy(ti32[:, sl], px[:, sl])
            nc.vector.tensor_copy(x0[:, sl], ti32[:, sl])
            nc.vector.tensor_tensor(ta[:, sl], x0[:, sl], px[:, sl], AL.is_gt)
            nc.vector.tensor_tensor(x0[:, sl], x0[:, sl], ta[:, sl], AL.subtract)
            nc.vector.tensor_copy(ti32[:, sl], py[:, sl])
            nc.vector.tensor_copy(y0[:, sl], ti32[:, sl])
            nc.vector.tensor_tensor(ta[:, sl], y0[:, sl], py[:, sl], AL.is_gt)
            nc.vector.tensor_tensor(y0[:, sl], y0[:, sl], ta[:, sl], AL.subtract)

            # px/py become fx/fy in place
            nc.vector.tensor_tensor(px[:, sl], px[:, sl], x0[:, sl], AL.subtract)
            nc.vector.tensor_tensor(py[:, sl], py[:, sl], y0[:, sl], AL.subtract)
            fx, fy = px, py

            # wy0m = (1-fy)*vy0*mask ; wy1m = fy*vy1*mask
            nc.vector.tensor_scalar(ta[:, sl], y0[:, sl], 0.0, None, AL.is_ge)
            nc.vector.tensor_scalar(tb[:, sl], y0[:, sl], 63.0, None, AL.is_le)
            nc.vector.tensor_tensor(ta[:, sl], ta[:, sl], tb[:, sl], AL.mult)
            nc.vector.tensor_tensor(ta[:, sl], ta[:, sl], mask[:, sl], AL.mult)
            nc.vector.tensor_scalar(tb[:, sl], fy[:, sl], -1.0, 1.0,
                                    AL.mult, AL.add)
            nc.vector.tensor_tensor(wy0m[:, sl], tb[:, sl], ta[:, sl], AL.mult)
            nc.vector.tensor_scalar(ta[:, sl], y0[:, sl], -1.0, None, AL.is_ge)
            nc.vector.tensor_scalar(tb[:, sl], y0[:, sl], 62.0, None, AL.is_le)
            nc.vector.tensor_tensor(ta[:, sl], ta[:, sl], tb[:, sl], AL.mult)
            nc.vector.tensor_tensor(ta[:, sl], ta[:, sl], mask[:, sl], AL.mult)
            nc.vector.tensor_tensor(wy1m[:, sl], fy[:, sl], ta[:, sl], AL.mult)

            # x slots
            nc.vector.tensor_scalar(xp[:, sl], x0[:, sl], 0.0, 62.0,
                                    AL.max, AL.min)
            nc.vector.tensor_tensor(ta[:, sl], x0[:, sl], xp[:, sl], AL.subtract)
            nc.vector.tensor_scalar(tb[:, sl], x0[:, sl], 0.0, None, AL.is_ge)
            nc.vector.tensor_scalar(tc_[:, sl], x0[:, sl], 63.0, None, AL.is_le)
            nc.vector.tensor_tensor(tb[:, sl], tb[:, sl], tc_[:, sl], AL.mult)
            nc.vector.tensor_scalar(tc_[:, sl], fx[:, sl], -1.0, 1.0,
                                    AL.mult, AL.add)
            nc.vector.tensor_tensor(tc_[:, sl], tc_[:, sl], tb[:, sl], AL.mult)
            nc.vector.tensor_scalar(tb[:, sl], x0[:, sl], -1.0, None, AL.is_ge)
            nc.vector.tensor_scalar(td[:, sl], x0[:, sl], 62.0, None, AL.is_le)
            nc.vector.tensor_tensor(tb[:, sl], tb[:, sl], td[:, sl], AL.mult)
            nc.vector.tensor_tensor(td[:, sl], fx[:, sl], tb[:, sl], AL.mult)

            nc.vector.tensor_scalar(tb[:, sl], ta[:, sl], 0.0, None, AL.is_equal)
            nc.vector.tensor_tensor(cs0[:, sl], tb[:, sl], tc_[:, sl], AL.mult)
            nc.vector.tensor_tensor(cs1[:, sl], tb[:, sl], td[:, sl], AL.mult)
            nc.vector.tensor_scalar(tb[:, sl], ta[:, sl], -1.0, None, AL.is_equal)
            nc.vector.tensor_tensor(tb[:, sl], tb[:, sl], td[:, sl], AL.mult)
            nc.vector.tensor_tensor(cs0[:, sl], cs0[:, sl], tb[:, sl], AL.add)
            nc.vector.tensor_scalar(tb[:, sl], ta[:, sl], 1.0, None, AL.is_equal)
            nc.vector.tensor_tensor(tb[:, sl], tb[:, sl], tc_[:, sl], AL.mult)
            nc.vector.tensor_tensor(cs1[:, sl], cs1[:, sl], tb[:, sl], AL.add)

            # pair-gather index: idx = clamp(y0+1, 0, 64)*64 + xp (fp32, exact)
            nc.vector.tensor_scalar(tb[:, sl], y0[:, sl], 1.0, 64.0,
                                    AL.add, AL.min)
            nc.vector.tensor_scalar(tb[:, sl], tb[:, sl], 0.0, None, AL.max)
            nc.vector.scalar_tensor_tensor(vidx[:, sl], tb[:, sl], 64.0,
                                           xp[:, sl], AL.mult, AL.add)

            # ---- wrapped-index build (PE transposes; fp32 stays exact) ----
            # vidx [128(p), 72(b*9+k)] -> gidx_cnk [128(16g+q), b''*72+k*8+r]
            # with p = 16r+q; g replicates across the 8 gpsimd core groups.
            ps1 = p_ps_ix.tile([72, 128], F32, tag="ps1", name="ps1")
            nc.tensor.transpose(ps1[:], vidx[:, sl], id128_sb[:])
            VR = p_head.tile([72, 8, 128], F32, tag="VR", name="VR")
            nc.vector.tensor_copy(
                _ap(VR[:], 0,
                    [[8 * 128, 72], [128, 8], [16, 8], [1, 16]]),
                _ap(ps1[:], 0, [[128, 72], [16, 8], [0, 8], [1, 16]]))
            for r in range(8):
                ps2 = p_ps_ix.tile([128, 72], F32, tag="pscm", name="pscm")
                nc.tensor.transpose(ps2[:], VR[:, r, :], id72)
                nc.vector.tensor_copy(
                    _ap(gidxs[cnk][:], r, [[8 * 72, 128], [72, 8], [8, 9]]),
                    _ap(ps2[:], 0, [[72, 128], [9, 8], [1, 9]]))

        xT2_src = _ap(d_xT2.ap(), 0, [[512, NROW2 - 1], [1, 1024]])

        # ---------------- main loop over 128-position blocks ----------------
        gctr = 0
        for h in range(NHT):
            gall = p_gall.tile([128, 18, 2, 128], BF, tag="gall", name="gall")
            for pb in range(2):
                b = 2 * h + pb
                # G [128(pos), 9(k), 1024(xs*512 + yc*256 + c)]
                G = p_G.tile([128, 9, 1024], BF, tag="G", name="G")
                for ci in range(3):
                    nc.gpsimd.dma_gather(
                        G[:, 3 * ci:3 * (ci + 1), :], xT2_src,
                        gidxs[b // 8][:, b % 8, 24 * ci:24 * (ci + 1)],
                        384, 384, 1024, elem_step=512, queue_num=gctr % 4)
                    gctr += 1
                # fused weighted pair-adds (custom DVE op, out = a*s0 + b*s1):
                # xs stage: H[yc,c] = cs0*G[xs0] + cs1*G[xs1] per k
                # yc stage: gT[k,c] = wy0m*H[yc0] + wy1m*H[yc1] per k
                gT = p_gT.tile([128, 9 * 256], BF, tag="gT", name="gT")
                for k in range(9):
                    col = b * 9 + k
                    nc.vector._custom_dve(
                        WPAIR, out=G[:, k, 0:512],
                        in0=G[:, k, 0:512], in1=G[:, k, 512:1024],
                        s0=cs0[:, col:col + 1], s1=cs1[:, col:col + 1])
                    nc.vector._custom_dve(
                        WPAIR, out=gT[:, k * 256:(k + 1) * 256],
                        in0=G[:, k, 0:256], in1=G[:, k, 256:512],
                        s0=wy0m[:, col:col + 1], s1=wy1m[:, col:col + 1])
                nc.sync.dma_start_transpose(
                    _ap(gall[:], pb * 128,
                        [[18 * 2 * 128, 128], [256, 18], [1, 128]]),
                    gT[:])
            # DCN matmul + BN1+ReLU into bands
            for ohalf in range(2):
                ps = p_ps_dcn.tile([128, 256], F32, tag="psdcn", name="psdcn")
                for j in range(18):
                    lhsT = _ap(wdcn_sb[:], j * 256 + ohalf * 128,
                               [[18 * 256, 128], [1, 128]])
                    nc.tensor.matmul(ps[:], lhsT, gall[:, j, :, :],
                                     start=(j == 0), stop=(j == 17))
                td0 = h // 2
                loc0 = 4 * (h % 2) + 1
                bb = bn1_sb[:, 1, ohalf:ohalf + 1]
                ss = bn1_sb[:, 0, ohalf:ohalf + 1]
                nc.scalar.activation(
                    _ap(bands[:], td0 * 1320 + ohalf * 660 + loc0 * 66 + 1,
                        [[8 * 2 * 660, 128], [66, 4], [1, 64]]),
                    ps[:], AF.Relu, bias=bb, scale=ss)
                if h % 2 == 0 and td0 > 0:
                    nc.scalar.activation(
                        _ap(bands[:], (td0 - 1) * 1320 + ohalf * 660 + 9 * 66 + 1,
                            [[8 * 2 * 660, 128], [1, 64]]),
                        ps[:, 0:64], AF.Relu, bias=bb, scale=ss)
                if h % 2 == 1 and td0 < 7:
                    nc.scalar.activation(
                        _ap(bands[:], (td0 + 1) * 1320 + ohalf * 660 + 1,
                            [[8 * 2 * 660, 128], [1, 64]]),
                        ps[:, 192:256], AF.Relu, bias=bb, scale=ss)

            # deconv for ready band
            td_ = None
            if h >= 2 and h % 2 == 0:
                td_ = h // 2 - 1
            elif h == NHT - 1:
                td_ = 7
            if td_ is None:
                continue
            for ohalf in range(2):
                outst = p_outst.tile([128, 2048], BF, tag="outst", name="outst")
                for par in range(4):
                    a, b_ = par // 2, par % 2
                    tap_y = TAP0 if a == 0 else TAP1
                    tap_x = TAP0 if b_ == 0 else TAP1
                    ps = p_ps_dc.tile([128, 512], F32, tag="psdc", name="psdc")
                    for j8 in range(8):
                        ti, tj, chalf = j8 // 4, (j8 // 2) % 2, j8 % 2
                        dr, ds = tap_y[ti][1], tap_x[tj][1]
                        lhsT = _ap(wup_sb[:],
                                   par * 8 * 256 + j8 * 256 + ohalf * 128,
                                   [[4 * 8 * 256, 128], [1, 128]])
                        rhs = _ap(bands[:],
                                  td_ * 1320 + chalf * 660 + (1 + dr) * 66 + 1 + ds,
                                  [[8 * 2 * 660, 128], [66, 8], [1, 64]])
                        nc.tensor.matmul(ps[:], lhsT, rhs,
                                         start=(j8 == 0), stop=(j8 == 7))
                    nc.scalar.activation(
                        _ap(outst[:], a * 128 + b_,
                            [[2048, 128], [256, 8], [2, 64]]),
                        ps[:], AF.Relu,
                        bias=bn2_sb[:, 1, ohalf:ohalf + 1], scale=bn2_sb[:, 0, ohalf:ohalf + 1])
                nc.sync.dma_start(
                    _ap(d_out.ap(), ohalf * 128 * 16384 + td_ * 16 * 128,
                        [[16384, 128], [1, 2048]]),
                    outst[:])

    nc.compile()
    return nc


# ---------------- host prep ----------------
def _prep_shared(inputs):
    w_off = np.asarray(inputs["w_off"], np.float32)
    b_off = np.asarray(inputs["b_off"], np.float32)
    w_dcn = np.asarray(inputs["w_dcn"], np.float32)
    w_up = np.asarray(inputs["w_up"], np.float32)

    woff = np.zeros((9, 2, 128, 27), np.float32)
    for k in range(9):
        for cb in range(2):
            woff[k, cb] = w_off[:, cb * 128:(cb + 1) * 128, k // 3, k % 3].T
    wdcn = np.zeros((128, 18, 256), np.float32)
    wd = w_dcn.reshape(Co, C, 9)
    for k in range(9):
        for chalf in range(2):
            wdcn[:, k * 2 + chalf, :] = wd[:, chalf * 128:(chalf + 1) * 128, k].T
    wup = np.zeros((128, 4, 8, 256), np.float32)
    for par in range(4):
        a, b_ = par // 2, par % 2
        tap_y = TAP0 if a == 0 else TAP1
        tap_x = TAP0 if b_ == 0 else TAP1
        for j8 in range(8):
            ti, tj, chalf = j8 // 4, (j8 // 2) % 2, j8 % 2
            kh, kw = tap_y[ti][0], tap_x[tj][0]
            # lhsT[p=c%128, o] = w_eff[o, c] = w_up[o, c, kh, kw]
            wup[:, par, j8, :] = w_up[:, chalf * 128:(chalf + 1) * 128, kh, kw].T

    ky = np.repeat(np.arange(3) - 1, 3).astype(np.float32)
    kx = np.tile(np.arange(3) - 1, 3).astype(np.float32)
    pos = np.arange(HW)
    hh = (pos // W).astype(np.float32)
    ww = (pos % W).astype(np.float32)
    FBp = np.zeros((HW, 27), np.float32)
    FBp[:, 0:9] = ww[:, None] + kx[None, :]
    FBp[:, 9:18] = hh[:, None] + ky[None, :]
    FBp += b_off[None, :]
    FB = np.ascontiguousarray(FBp.reshape(NBLK, 128, 27).transpose(1, 0, 2))

    def bnfold(g, b, m, v):
        s = np.asarray(g) / np.sqrt(np.asarray(v) + EPS)
        return s.astype(np.float32), (np.asarray(b) - np.asarray(m) * s).astype(np.float32)

    s1, b1 = bnfold(inputs["bn1_g"], inputs["bn1_b"], inputs["bn1_m"], inputs["bn1_v"])
    s2, b2 = bnfold(inputs["bn2_g"], inputs["bn2_b"], inputs["bn2_m"], inputs["bn2_v"])
    bn1 = np.stack([s1.reshape(2, 128), b1.reshape(2, 128)])
    bn2 = np.stack([s2.reshape(2, 128), b2.reshape(2, 128)])

    return dict(
        woff=woff.astype(BF16), wdcn=wdcn.astype(BF16), wup=wup.astype(BF16),
        FB=FB.astype(np.float32), bn1=bn1.astype(np.float32),
        bn2=bn2.astype(np.float32), id27=np.eye(27, dtype=np.float32),
        id128=np.eye(128, dtype=np.float32),
    )


def _prep_sample(xb):
    xb = np.asarray(xb, np.float32)
    xpad = np.zeros((C, 66, 66), np.float32)
    xpad[:, 1:65, 1:65] = xb.reshape(C, 64, 64)
    xpad = xpad.reshape(2, 128, PADHW)
    # xT2: row q (q = (y+1)*64 + x) = [xrow(y, x), xrow(y+1, x)], zeros
    # outside [0, 63]; one 1024-elem window at 512*q covers the 2x2 patch.
    xr = xb.reshape(C, HW).T  # [pos, C]
    xT2 = np.zeros((NROW2, 2, C), np.float32)
    xT2[64:, 0, :] = xr                  # slot0: xrow(q-64)
    xT2[:HW, 1, :] = xr                  # slot1: xrow(q)
    return dict(xpad=xpad.astype(BF16), xT2=xT2.reshape(-1).astype(BF16))


_NC_CACHE = {}
TRACE = False
LAST_RESULT = None


def kernel(**inputs):
    global LAST_RESULT
    if "nc" not in _NC_CACHE:
        _NC_CACHE["nc"] = build_nc()
    nc = _NC_CACHE["nc"]
    shared = _prep_shared(inputs)
    x = np.asarray(inputs["x"])
    in_maps = [dict(shared, **_prep_sample(x[b])) for b in range(x.shape[0])]
    res = bass_utils.run_bass_kernel_spmd(nc, in_maps, core_ids=list(range(8)),
                                          trace=TRACE)
    LAST_RESULT = res
    out = np.stack([res.results[b]["out"] for b in range(len(in_maps))])
    return out.astype(np.float32)
